# revision 16
# baseline (speedup 1.0000x reference)
"""KA-GNN (Fourier-KAN message passing) on 8 Trainium2 NeuronCores.

Sharding: nodes/edges partitioned by destination range across 8 cores.
Per conv layer each core computes its msg shard (node-wise Fourier-KAN),
an AllGather builds the full msg table in DRAM, then per 128-dst window
dma_gather pulls source rows and a one-hot scatter-matmul (PSUM
accumulation) performs the segment sum. Pool via one-hot matmul + small
AllReduce; readout + sigmoid on device.

The Fourier features sin/cos(k*h), k=1..4 are built from sin(h), cos(h)
(range-reduced via round-to-nearest f32->i32 cast) plus ScalarE Square
chains; the k-harmonics are linear in 8 basis tensors, so the KAN weights
are remixed host-side onto that basis (plus a per-output bias column).
"""

import math
import numpy as np
import ml_dtypes

import concourse.bass as bass
import concourse.bacc as bacc
import concourse.mybir as mybir
import concourse.tile as tile
from concourse.bass_utils import run_bass_kernel_spmd

F32 = mybir.dt.float32
BF16 = mybir.dt.bfloat16
I16 = mybir.dt.int16
I32 = mybir.dt.int32
AF = mybir.ActivationFunctionType
OP = mybir.AluOpType

P = 8
HID = 32
INF = 64
NG = 128
NCONV = 2
NEG = 0.01

NPC = 6656                 # nodes per core (padded total 53248)
NTOT = NPC * P
NBLK = 4
BLK = NPC // NBLK          # 1664
WIN = 128
NWIN = NPC // WIN          # 52
WQ = 4                     # windows per gather batch ("quad")
NQ = NWIN // WQ            # 13
N_NODES_REAL = 50000

TWO_PI = float(2 * math.pi)
PI = float(math.pi)
INV_2PI = float(1.0 / (2 * math.pi))
ISQ2 = float(1.0 / math.sqrt(2.0))

LAST_RESULTS = None        # test.py reads exec_time_ns from here


def _install_ntff_hook():
    # restore the axon NTFF profiling hook when the image's antenv lacks it
    import sys
    import types
    try:
        import antenv.axon_hooks  # noqa: F401
        return
    except ImportError:
        pass
    try:
        import antenv
        from trn_agent_boot.trn_boot import _ntff_profile_via_ctypes
        hook = _ntff_profile_via_ctypes("/opt/axon/libaxon_pjrt.so")
        mod = types.ModuleType("antenv.axon_hooks")
        holder = {"h": hook}
        mod.set_axon_ntff_profile_hook = lambda h: holder.__setitem__("h", h)
        mod.get_axon_ntff_profile_hook = lambda: holder["h"]
        sys.modules["antenv.axon_hooks"] = mod
        antenv.axon_hooks = mod
    except Exception:
        pass


_install_ntff_hook()


# ----------------------------------------------------------------------------
# host-side sharding / index prep
# ----------------------------------------------------------------------------

def _prep(edge_index, batch):
    src = np.asarray(edge_index[0], dtype=np.int64)
    dst = np.asarray(edge_index[1], dtype=np.int64)
    bat = np.asarray(batch, dtype=np.int64)
    E = src.shape[0]

    core = dst // NPC
    w_in_core = (dst % NPC) // WIN
    dloc = dst % WIN
    run = (src % 2).astype(np.int64)          # which 32-col slice of gathered row
    gidx = (src // 2).astype(np.int64)        # table row (2 nodes per 256B row)

    key = (core * NWIN + w_in_core) * 2 + run
    cnt = np.bincount(key, minlength=P * NWIN * 2).reshape(P, NWIN, 2)
    KR0 = max(int(np.ceil(cnt[:, :, 0].max() / 128)), 1)
    KR1 = max(int(np.ceil(cnt[:, :, 1].max() / 128)), 1)
    NCPW = KR0 + KR1                          # chunks per window

    order = np.argsort(key, kind="stable")
    s_key = key[order]
    s_gidx = gidx[order]
    s_dloc = dloc[order]
    grp_start = np.zeros(P * NWIN * 2, dtype=np.int64)
    grp_start[1:] = np.cumsum(np.bincount(s_key, minlength=P * NWIN * 2))[:-1]
    pos = np.arange(E) - grp_start[s_key]

    cap0 = KR0 * 128
    tok_per_win = NCPW * 128
    tok_per_core = NWIN * tok_per_win

    s_core = s_key // (NWIN * 2)
    s_win = (s_key // 2) % NWIN
    s_run = s_key % 2
    slot = (s_core * tok_per_core + s_win * tok_per_win
            + np.where(s_run == 0, pos, cap0 + pos))

    tok_idx = np.zeros(P * tok_per_core, dtype=np.int16)      # pad -> row 0
    tok_dloc = np.full(P * tok_per_core, 255.0, dtype=np.float32)
    tok_idx[slot] = s_gidx.astype(np.int16)
    tok_dloc[slot] = s_dloc.astype(np.float32)

    # gather-idx layout per (core, quad): token t at [16g + t%16, t//16]
    ti = tok_idx.reshape(P, NQ, WQ * tok_per_win // 16, 16)
    ti = np.swapaxes(ti, 2, 3)                                # (P, NQ, 16, ntok/16)
    ti = np.concatenate([ti[:, q] for q in range(NQ)], axis=2)  # (P, 16, NQ*ntok/16)
    gidx_dev = np.tile(ti, (1, 8, 1)).copy()                  # (P, 128, ...)

    td = tok_dloc.reshape(P, NWIN * NCPW, 128)
    dloc_dev = np.ascontiguousarray(np.swapaxes(td, 1, 2))    # (P, 128, NWIN*NCPW)

    bat_pad = np.full(NTOT, -1, dtype=np.int64)
    bat_pad[:N_NODES_REAL] = bat
    B = (bat_pad[:, None] == np.arange(NG)[None, :])
    B_dev = np.ascontiguousarray(
        B.reshape(P, NWIN, 128, NG).transpose(0, 2, 1, 3)
        .reshape(P, 128, NWIN * NG)).astype(ml_dtypes.bfloat16)
    counts = np.bincount(bat, minlength=NG)[:NG].astype(np.float32)
    invc = (1.0 / np.maximum(counts, 1.0)).reshape(NG, 1)

    return dict(KR0=KR0, KR1=KR1, NCPW=NCPW,
                gidx_dev=gidx_dev, dloc_dev=dloc_dev, B_dev=B_dev, invc=invc)


def _pack_x(x):
    xp = np.zeros((NTOT, INF), dtype=np.float32)
    xp[:N_NODES_REAL] = x
    xc = xp.reshape(P, 2, NPC // 2, INF)
    return np.ascontiguousarray(xc.transpose(0, 1, 3, 2).reshape(P, 128, NPC // 2))


def _remix(W):
    """W: (2, out, in, 4) -> 8 slot matrices (out, in) + bias (out,).

    Basis slots: [sin h, cos h, sin^2 h, (1+sin2h)/2, sin^2 2h,
                  (1-sin4h)/2, sin3h, cos3h]."""
    W0, W1 = W[0], W[1]          # cos / sin coefficient stacks
    slots = [
        W1[:, :, 0],
        W0[:, :, 0],
        -2.0 * W0[:, :, 1],
        2.0 * W1[:, :, 1],
        -2.0 * W0[:, :, 3],
        -2.0 * W1[:, :, 3],
        W1[:, :, 2],
        W0[:, :, 2],
    ]
    bias = (W0[:, :, 1] - W1[:, :, 1] + W0[:, :, 3] + W1[:, :, 3]).sum(axis=1)
    return slots, bias.astype(np.float32)


def _pack_weights(W_in, W_conv, W_out):
    sl_in, b_in = _remix(W_in)
    win = np.zeros((128, 8 * HID), dtype=np.float32)
    for b in range(2):
        for m in range(8):
            win[64 * b:64 * b + 64, 32 * m:32 * m + 32] = sl_in[m].T
    wc = np.zeros((128, NCONV * 8 * HID), dtype=np.float32)
    biases = np.zeros((128, 1 + NCONV), dtype=np.float32)
    biases[:, 0] = np.tile(b_in, 4)
    for l in range(NCONV):
        sl, bl = _remix(W_conv[l])
        biases[:, 1 + l] = np.tile(bl, 4)
        for b in range(4):
            for m in range(8):
                wc[32 * b:32 * b + 32,
                   l * 8 * HID + 32 * m:l * 8 * HID + 32 * m + 32] = sl[m].T
    w0r = np.tile(W_out[0, 0, :, 0].astype(np.float32), (128, 1))
    w1r = np.tile(W_out[1, 0, :, 0].astype(np.float32), (128, 1))
    return win, wc, biases, w0r, w1r


# ----------------------------------------------------------------------------
# device program
# ----------------------------------------------------------------------------

def _build(meta):
    KR0, KR1, NCPW = meta["KR0"], meta["KR1"], meta["NCPW"]
    NTOK_Q = WQ * NCPW * 128
    XCOLS = NPC // 2                  # 3328

    nc = bacc.Bacc("TRN2", target_bir_lowering=False, debug=False,
                   num_devices=P, num_swdge_queues=4)

    x_d = nc.dram_tensor("x_pack", [128, XCOLS], F32, kind="ExternalInput")
    win_d = nc.dram_tensor("win_w", [128, 8 * HID], F32, kind="ExternalInput")
    wc_d = nc.dram_tensor("wc_w", [128, NCONV * 8 * HID], F32, kind="ExternalInput")
    bias_d = nc.dram_tensor("biases", [128, 1 + NCONV], F32, kind="ExternalInput")
    w0_d = nc.dram_tensor("w0r", [128, HID], F32, kind="ExternalInput")
    w1_d = nc.dram_tensor("w1r", [128, HID], F32, kind="ExternalInput")
    bout_d = nc.dram_tensor("bout", [128, 1], F32, kind="ExternalInput")
    invc_d = nc.dram_tensor("invc", [128, 1], F32, kind="ExternalInput")
    gidx_d = nc.dram_tensor("gidx", [128, NQ * NTOK_Q // 16], I16, kind="ExternalInput")
    dloc_d = nc.dram_tensor("dloc", [128, NWIN * NCPW], F32, kind="ExternalInput")
    iota_d = nc.dram_tensor("iota", [128, 128], F32, kind="ExternalInput")
    id32_d = nc.dram_tensor("id32", [128, 32], F32, kind="ExternalInput")
    id32b_d = nc.dram_tensor("id32b", [128, 32], BF16, kind="ExternalInput")
    B_d = nc.dram_tensor("Bmat", [128, NWIN * NG], BF16, kind="ExternalInput")

    out_d = nc.dram_tensor("out", [NG, 1], F32, kind="ExternalOutput")

    AG_GROUPS = [list(range(P))]
    NT = 416

    with tile.TileContext(nc) as tc:
        with (
            tc.tile_pool(name="const", bufs=1) as cp,
            tc.tile_pool(name="feat", bufs=1) as fp,
            tc.tile_pool(name="ftmp", bufs=1) as tp,
            tc.tile_pool(name="work", bufs=1) as wp,
            tc.tile_pool(name="gbuf", bufs=2) as gp,
            tc.tile_pool(name="ohp", bufs=2) as ohp,
            tc.tile_pool(name="pmsg", bufs=2, space="PSUM") as pmsg_p,
            tc.tile_pool(name="ptr", bufs=2, space="PSUM") as ptr_p,
            tc.tile_pool(name="pm", bufs=3, space="PSUM") as pm_p,
            tc.tile_pool(name="ppool", bufs=1, space="PSUM") as ppool_p,
            tc.tile_pool(name="dram", bufs=1, space="DRAM") as dp,
        ):
            # ---- constants ----
            x_sb = wp.tile([128, XCOLS], F32, name="x_sb", tag="bigx")
            nc.sync.dma_start(x_sb[:], x_d[:])
            win_sb = cp.tile([128, 8 * HID], F32)
            nc.sync.dma_start(win_sb[:], win_d[:])
            wc_sb = cp.tile([128, NCONV * 8 * HID], F32)
            nc.sync.dma_start(wc_sb[:], wc_d[:])
            bias_sb = cp.tile([128, 1 + NCONV], F32)
            nc.sync.dma_start(bias_sb[:], bias_d[:])
            w0_sb = cp.tile([128, HID], F32)
            nc.sync.dma_start(w0_sb[:], w0_d[:])
            w1_sb = cp.tile([128, HID], F32)
            nc.sync.dma_start(w1_sb[:], w1_d[:])
            bout_sb = cp.tile([128, 1], F32)
            nc.sync.dma_start(bout_sb[:], bout_d[:])
            invc_sb = cp.tile([128, 1], F32)
            nc.sync.dma_start(invc_sb[:], invc_d[:])
            gidx_sb = cp.tile([128, NQ * NTOK_Q // 16], I16)
            nc.sync.dma_start(gidx_sb[:], gidx_d[:])
            dloc_sb = cp.tile([128, NWIN * NCPW], F32)
            nc.sync.dma_start(dloc_sb[:], dloc_d[:])
            iota_sb = cp.tile([128, 128], F32)
            nc.sync.dma_start(iota_sb[:], iota_d[:])
            id32_sb = cp.tile([128, 32], F32)
            nc.sync.dma_start(id32_sb[:], id32_d[:])
            id32b_sb = cp.tile([128, 32], BF16)
            nc.sync.dma_start(id32b_sb[:], id32b_d[:])
            zb = cp.tile([128, 1], F32)
            nc.vector.memset(zb[:], 0.0)
            m1 = cp.tile([128, 1], F32)
            nc.vector.memset(m1[:], -1.0)

            h_sb = cp.tile([128, BLK], F32)    # packed h^T: partition 32*blk+f

            shard = [dp.tile([NPC, HID], F32, name=f"shard{l}") for l in range(NCONV)]
            table = [dp.tile([NTOT, HID], F32, name=f"table{l}", addr_space="Shared")
                     for l in range(NCONV)]
            pool_in = dp.tile([HID, NG], F32)
            pool_out = dp.tile([HID, NG], F32, addr_space="Shared")

            def feat_chain(src, FREE, pfx):
                """8 basis-feature f32 tiles of (128, FREE) from f32 src."""
                def ts(dst, a, s1, s2, o0, o1=None):
                    if o1 is None:
                        nc.vector.tensor_scalar(dst, a, s1, None, o0)
                    else:
                        nc.vector.tensor_scalar(dst, a, s1, s2, o0, o1)

                def scr(nm, dt=F32):
                    return tp.tile([128, FREE], dt, name=f"{pfx}{nm}", tag="scr",
                                   bufs=3, padded_shape=[128, BLK])

                slots = [fp.tile([128, FREE], F32, name=f"{pfx}slot{i}",
                                 tag=f"feat{i}", padded_shape=[128, BLK])
                         for i in range(8)]
                s1f, c1f, sqsf, sqdf = slots[0], slots[1], slots[2], slots[3]

                n0 = scr("n0", I32)
                ts(n0[:], src, INV_2PI, None, OP.mult)
                nf0 = scr("nf0")
                ts(nf0[:], n0[:], -TWO_PI, None, OP.mult)
                r0 = scr("r0")
                nc.vector.tensor_tensor(r0[:], src, nf0[:], OP.add)
                n9 = scr("n9", I32)
                ts(n9[:], src, INV_2PI, 0.25, OP.mult, OP.add)
                nf9 = scr("nf9")
                ts(nf9[:], n9[:], -TWO_PI, PI / 2, OP.mult, OP.add)
                r9 = scr("r9")
                nc.vector.tensor_tensor(r9[:], src, nf9[:], OP.add)

                nc.scalar.activation(s1f[:], r0[:], AF.Sin, bias=zb[:])
                nc.scalar.activation(c1f[:], r9[:], AF.Sin, bias=zb[:])
                nc.scalar.activation(sqsf[:], s1f[:], AF.Square)
                d = scr("d")
                nc.vector.tensor_tensor(d[:], s1f[:], c1f[:], OP.add)
                nc.scalar.activation(sqdf[:], d[:], AF.Square, scale=ISQ2)
                nc.scalar.activation(slots[4][:], sqdf[:], AF.Square,
                                     bias=m1[:], scale=2.0)
                tc2 = scr("tc2")
                ts(tc2[:], sqsf[:], -2.0, 1.0, OP.mult, OP.add)
                ts2 = scr("ts2")
                ts(ts2[:], sqdf[:], 2.0, -1.0, OP.mult, OP.add)
                td2 = scr("td2")
                nc.vector.tensor_tensor(td2[:], tc2[:], ts2[:], OP.subtract)
                nc.scalar.activation(slots[5][:], td2[:], AF.Square, scale=ISQ2)
                t3 = scr("t3")
                ts(t3[:], sqsf[:], -4.0, 3.0, OP.mult, OP.add)
                nc.vector.tensor_tensor(slots[6][:], s1f[:], t3[:], OP.mult)
                t4 = scr("t4")
                ts(t4[:], sqsf[:], -4.0, 1.0, OP.mult, OP.add)
                nc.vector.tensor_tensor(slots[7][:], c1f[:], t4[:], OP.mult)
                return slots

            # ================= input KAN: x -> h =================
            for half in range(2):
                xsl = x_sb[:, half * BLK:(half + 1) * BLK]
                slots = feat_chain(xsl, BLK, f"x{half}_")
                for b2 in range(2):
                    for t4i in range(BLK // NT):
                        node0 = b2 * XCOLS + half * BLK + NT * t4i
                        hb, off = node0 // BLK, node0 % BLK
                        ph = pmsg_p.tile([128, NT], F32, name="ph", tag="pmsg")
                        phs = ph[32 * hb:32 * hb + 32, :]
                        for m in range(8):
                            nc.tensor.matmul(
                                phs,
                                win_sb[64 * b2:64 * b2 + 64, 32 * m:32 * m + 32],
                                slots[m][64 * b2:64 * b2 + 64, NT * t4i:NT * (t4i + 1)],
                                start=(m == 0), stop=(m == 7),
                                tile_position=(64 * b2, 32 * hb),
                            )
                        nc.vector.tensor_scalar(
                            h_sb[32 * hb:32 * hb + 32, off:off + NT], phs,
                            bias_sb[32 * hb:32 * hb + 32, 0:1], None, OP.add)

            # ================= conv layers =================
            for l in range(NCONV):
                slots = feat_chain(h_sb[:], BLK, f"c{l}_")

                mTp = wp.tile([128, BLK], F32, name=f"mT{l}", tag="mT", bufs=2)
                for b in range(NBLK):
                    for t4i in range(BLK // NT):
                        pm2 = pmsg_p.tile([128, NT], F32, name="pm2", tag="pmsg")
                        pm2s = pm2[32 * b:32 * b + 32, :]
                        for m in range(8):
                            nc.tensor.matmul(
                                pm2s,
                                wc_sb[32 * b:32 * b + 32,
                                      l * 8 * HID + 32 * m:l * 8 * HID + 32 * m + 32],
                                slots[m][32 * b:32 * b + 32, NT * t4i:NT * (t4i + 1)],
                                start=(m == 0), stop=(m == 7),
                                tile_position=(32 * b, 32 * b),
                            )
                        nc.vector.tensor_scalar(
                            mTp[32 * b:32 * b + 32, NT * t4i:NT * (t4i + 1)], pm2s,
                            bias_sb[32 * b:32 * b + 32, 1 + l:2 + l], None, OP.add)
                for b in range(NBLK):
                    stage = wp.tile([128, 13 * HID], F32, name=f"stg{l}_{b}",
                                    tag="stage", bufs=2)
                    for w2 in range(13):
                        ptr = ptr_p.tile([128, 32], F32, name="ptrt", tag="ptr")
                        nc.tensor.transpose(
                            ptr[:], mTp[32 * b:32 * b + 32, 128 * w2:128 * (w2 + 1)],
                            id32_sb[32 * b:32 * b + 32, :],
                            tile_position=(32 * b, 0))
                        nc.vector.tensor_copy(stage[:, 32 * w2:32 * (w2 + 1)], ptr[:])
                    nc.sync.dma_start(
                        shard[l][BLK * b:BLK * (b + 1), :].rearrange(
                            "(w p) f -> p w f", p=128),
                        stage[:].rearrange("p (w f) -> p w f", f=HID),
                    )

                nc.gpsimd.collective_compute(
                    "AllGather", OP.bypass,
                    ins=[shard[l][:]], outs=[table[l][:]],
                    replica_groups=AG_GROUPS,
                )
                tab_ap = table[l][:].rearrange("(n two) f -> n (two f)", two=2)

                for q in range(NQ):
                    G = gp.tile([128, WQ * NCPW, 64], F32, name="G", tag="G")
                    nc.gpsimd.dma_gather(
                        G[:], tab_ap,
                        gidx_sb[:, q * (NTOK_Q // 16):(q + 1) * (NTOK_Q // 16)],
                        num_idxs=NTOK_Q, num_idxs_reg=NTOK_Q,
                        elem_size=64, single_packet=False, queue_num=q % 4,
                    )
                    iota_b = iota_sb[:].rearrange("p (x d) -> p x d", x=1)
                    for wi in range(WQ):
                        w = q * WQ + wi
                        hb, off = (w * WIN) // BLK, (w * WIN) % BLK
                        pm = pm_p.tile([128, WIN], F32, name="pmw", tag="pm")
                        pms = pm[32 * hb:32 * hb + 32, :]
                        oh = ohp.tile([128, NCPW, WIN], F32, name="oh", tag="oh")
                        half = (NCPW + 1) // 2
                        for hseg in range(2):
                            c0 = hseg * half
                            c1 = min(NCPW, c0 + half)
                            if c0 >= c1:
                                continue
                            nc.vector.tensor_tensor(
                                oh[:, c0:c1, :],
                                iota_b.to_broadcast([128, c1 - c0, WIN]),
                                dloc_sb[:, w * NCPW + c0:w * NCPW + c1]
                                .to_broadcast([128, c1 - c0, WIN]),
                                OP.is_equal)
                        for c in range(NCPW):
                            roff = 0 if c < KR0 else 32
                            nc.tensor.matmul(
                                pms, G[:, wi * NCPW + c, roff:roff + 32],
                                oh[:, c, :],
                                start=(c == 0), stop=(c == NCPW - 1),
                                tile_position=(0, 32 * hb),
                            )
                        hsl = h_sb[32 * hb:32 * hb + 32, off:off + WIN]
                        nc.vector.tensor_tensor(hsl, pms, hsl, OP.add)
                # leaky relu on the whole packed tile: h = max(z, 0.01*z)
                lrt = wp.tile([128, BLK], F32, name=f"lr{l}", tag="lrt", bufs=1)
                nc.vector.tensor_scalar(lrt[:], h_sb[:], NEG, None, OP.mult)
                nc.vector.tensor_tensor(h_sb[:], h_sb[:], lrt[:], OP.max)

            # ================= pool + readout =================
            B_sb = wp.tile([128, NWIN * NG], BF16, name="B_sb", tag="bigx")
            nc.sync.dma_start(B_sb[:], B_d[:])
            ppool = ppool_p.tile([HID, NG], F32)
            hbf = wp.tile([128, BLK], BF16, name="hbf", tag="hbf", bufs=1)
            nc.vector.tensor_copy(hbf[:], h_sb[:])
            for b in range(NBLK):
                for w2 in range(13):
                    w = 13 * b + w2
                    ptb = ptr_p.tile([128, 32], BF16, name="ptb", tag="ptr")
                    nc.tensor.transpose(ptb[:],
                                        hbf[32 * b:32 * b + 32, 128 * w2:128 * (w2 + 1)],
                                        id32b_sb[32 * b:32 * b + 32, :],
                                        tile_position=(32 * b, 0))
                    htile = wp.tile([128, 32], BF16, name="htile", tag="htile", bufs=3)
                    nc.vector.tensor_copy(htile[:], ptb[:])
                    nc.tensor.matmul(
                        ppool[:], htile[:], B_sb[:, NG * w:NG * (w + 1)],
                        start=(w == 0), stop=(w == NWIN - 1),
                    )
            pool_sb = wp.tile([HID, NG], F32, name="pool_sb")
            nc.vector.tensor_copy(pool_sb[:], ppool[:])
            nc.sync.dma_start(pool_in[:], pool_sb[:])
            nc.gpsimd.collective_compute(
                "AllReduce", OP.add,
                ins=[pool_in[:]], outs=[pool_out[:]],
                replica_groups=AG_GROUPS,
            )
            psum_sb = wp.tile([HID, NG], F32, name="psum_sb")
            nc.sync.dma_start(psum_sb[:], pool_out[:])
            ptry = ptr_p.tile([128, 32], F32, name="ptry", tag="ptr")
            nc.tensor.transpose(ptry[:], psum_sb[:], id32_sb[0:32, :])
            y_sb = wp.tile([NG, HID], F32, name="y_sb")
            nc.vector.tensor_scalar(y_sb[:], ptry[:], invc_sb[:], None, OP.mult)

            # readout: sin(y), cos(y) via the same range reduction
            def sincos(src, pfx, quarter):
                n = wp.tile([NG, HID], I32, name=f"{pfx}n")
                nf = wp.tile([NG, HID], F32, name=f"{pfx}nf")
                if quarter:
                    nc.vector.tensor_scalar(n[:], src, INV_2PI, 0.25, OP.mult, OP.add)
                    nc.vector.tensor_scalar(nf[:], n[:], -TWO_PI, PI / 2,
                                            OP.mult, OP.add)
                else:
                    nc.vector.tensor_scalar(n[:], src, INV_2PI, None, OP.mult)
                    nc.vector.tensor_scalar(nf[:], n[:], -TWO_PI, None, OP.mult)
                r = wp.tile([NG, HID], F32, name=f"{pfx}r")
                nc.vector.tensor_tensor(r[:], src, nf[:], OP.add)
                o = wp.tile([NG, HID], F32, name=f"{pfx}o")
                nc.scalar.activation(o[:], r[:], AF.Sin, bias=zb[:])
                return o

            sin_y = sincos(y_sb[:], "sy", False)
            cos_y = sincos(y_sb[:], "cy", True)
            nc.vector.tensor_tensor(cos_y[:], cos_y[:], w0_sb[:], OP.mult)
            nc.vector.tensor_tensor(sin_y[:], sin_y[:], w1_sb[:], OP.mult)
            nc.vector.tensor_tensor(cos_y[:], cos_y[:], sin_y[:], OP.add)
            red = wp.tile([NG, 1], F32, name="red")
            nc.vector.tensor_reduce(red[:], cos_y[:], mybir.AxisListType.X, OP.add)
            o_sb = wp.tile([NG, 1], F32, name="o_sb")
            nc.scalar.activation(o_sb[:], red[:], AF.Sigmoid, bias=bout_sb[:])
            nc.sync.dma_start(out_d[:], o_sb[:])

    nc.compile()
    return nc


# ----------------------------------------------------------------------------
# entry point
# ----------------------------------------------------------------------------

def kernel(x, edge_index, batch, W_in, W_conv, W_out, b_out):
    global LAST_RESULTS
    x = np.asarray(x, dtype=np.float32)
    W_in = np.asarray(W_in, dtype=np.float32)
    W_conv = np.asarray(W_conv, dtype=np.float32)
    W_out = np.asarray(W_out, dtype=np.float32)
    b_out = np.asarray(b_out, dtype=np.float32)

    meta = _prep(edge_index, batch)
    x_pack = _pack_x(x)
    win, wc, biases, w0r, w1r = _pack_weights(W_in, W_conv, W_out)

    nc = _build(meta)

    iota = np.tile(np.arange(128, dtype=np.float32)[None, :], (128, 1))
    id32 = np.tile(np.eye(32, dtype=np.float32), (4, 1))
    id32b = np.tile(np.eye(32, dtype=ml_dtypes.bfloat16), (4, 1))
    bout_col = np.full((128, 1), float(b_out.ravel()[0]), dtype=np.float32)

    in_maps = []
    for c in range(P):
        in_maps.append({
            "x_pack": x_pack[c],
            "win_w": win,
            "wc_w": wc,
            "biases": biases,
            "w0r": w0r,
            "w1r": w1r,
            "bout": bout_col,
            "invc": meta["invc"].astype(np.float32),
            "gidx": meta["gidx_dev"][c],
            "dloc": meta["dloc_dev"][c],
            "iota": iota,
            "id32": id32,
            "id32b": id32b,
            "Bmat": meta["B_dev"][c],
        })

    res = run_bass_kernel_spmd(nc, in_maps, core_ids=list(range(P)))
    LAST_RESULTS = res
    return np.asarray(res.results[0]["out"], dtype=np.float32)
